# revision 5
# baseline (speedup 1.0000x reference)
"""Causal MHA (batch=4, seq=2048, dim=1024, 16 heads x 64) on 8 TRN2 NeuronCores.

Sharding: core c handles batch b = c//2 and head-group g = c%2 (8 heads).
Each core computes QKV projections for its heads, causal attention, and a
partial output projection over its 512 features. The host sums the two
partial projections per batch and transposes back.

v2 schedule: software-pipelined weave. PV matmuls of unit u-1 (a unit is
one (q-chunk, head-pair)) are interleaved into the S matmuls of unit u so
the PE never waits on the exp chain; softmax normalization lags one unit
(reciprocal on DVE, broadcast via rank-1 matmul). Diagonal S/PV matmuls
are narrowed to the causally valid columns. Input DMAs ride the two idle
HWDGE queues (sync + act); dummy warm-up matmuls hold the HAM clock gate
at 2.4 GHz through the DMA-paced head.
"""
import sys

sys.path.insert(0, "/opt/trn_rl_repo")

import json
import numpy as np
import ml_dtypes
from contextlib import ExitStack

import concourse.bass as bass
import concourse.tile as tile
from concourse import mybir
from concourse.bass_utils import run_bass_kernel_spmd

BF16 = mybir.dt.bfloat16
F32 = mybir.dt.float32
Exp = mybir.ActivationFunctionType.Exp

DIM = 1024
SEQ = 2048
NH = 16          # total heads
HPC = 8          # heads per core
DH = 64          # head dim
SCALE = DH ** -0.5
NCORES = 8
FPC = HPC * DH   # features per core = 512
NKT = SEQ // 128   # 16 k-tiles of 128
NQC = SEQ // 512   # 4 q-chunks of 512
VSTRIDE = DH + 2   # 66: V columns per head incl. ones col + pad

_WALRUS_PATCHED = False


def _patch_walrus_wait_limit():
    """This container's walrus rejects >1 sem wait per instruction
    (CoreV3 setupSyncWait). Tile's tail drain carries one wait per live
    proc; split the extras into preceding single-wait carriers at
    BIR-JSON serialization time. Also merge row-tiled Ldweights pairs
    and drop Ldweights that reload already-resident weights."""
    global _WALRUS_PATCHED
    if _WALRUS_PATCHED:
        return
    _WALRUS_PATCHED = True

    orig = bass.Bass.to_json_bytes

    def _merge_ldw_halves(insts):
        """Fold row-tiled Ldweights pairs ([64,128] at row 0 + [64,128] at
        row 64 of the same tensor) into one [128,128] load carrying both
        halves' waits."""
        out = []
        pend = None  # (index_in_out, inst) of a candidate row-0 half
        for inst in insts:
            op = inst["opcode"]
            if inst.get("engine") != "PE":
                out.append(inst)
                continue
            if op == "Ldweights" and inst.get("tile_size") == [64, 128]:
                ap = inst["ins"][0].get("ap")
                if inst.get("tile_position") == [0, 0] and ap and ap[0][1] == 64:
                    out.append(inst)
                    pend = (len(out) - 1, inst)
                    continue
                if (pend is not None
                        and inst.get("tile_position") == [64, 0] and ap
                        and ap[0][1] == 64):
                    a = pend[1]
                    aap = a["ins"][0]["ap"]
                    same = (a["ins"][0].get("memref") == inst["ins"][0].get("memref")
                            and aap[0][0] == ap[0][0] and aap[1] == ap[1]
                            and inst["ins"][0].get("offset", 0)
                            == a["ins"][0].get("offset", 0) + 64 * aap[0][0])
                    b_si = inst.get("sync_info") or {}
                    if same and not b_si.get("on_update"):
                        aap[0][1] = 128
                        a["tile_size"] = [128, 128]
                        a.setdefault("sync_info", {"on_update": [], "on_wait": []})
                        a["sync_info"].setdefault("on_wait", [])
                        a["sync_info"]["on_wait"].extend(b_si.get("on_wait") or [])
                        pend = None
                        continue
                out.append(inst)
                pend = None
            else:
                if op not in ("Matmult", "NoOp"):
                    pend = None
                out.append(inst)
        return out

    def patched(self, *a, **k):
        d = json.loads(orig(self, *a, **k))
        for f in d["functions"]:
            for bb in f["blocks"]:
                bb["instructions"] = _merge_ldw_halves(bb["instructions"])
                out = []
                last_ldw = None
                for inst in bb["instructions"]:
                    si = inst.get("sync_info")
                    ow = (si or {}).get("on_wait") or []
                    op = inst["opcode"]

                    def emit_carriers(waits):
                        for j, w in enumerate(waits):
                            out.append({
                                "name": f"{inst['name']}__w{j}",
                                "opcode": "NoOp",
                                "engine": inst["engine"],
                                "ins": [], "outs": [],
                                "debug": inst.get("debug", 0),
                                "sync_info": {"on_update": [], "on_wait": [w]},
                            })

                    if op == "Ldweights" and inst["engine"] == "PE":
                        key = json.dumps(
                            [inst.get("ins"), inst.get("tile_position"),
                             inst.get("tile_size")], sort_keys=True)
                        if last_ldw == key and not (si or {}).get("on_update"):
                            emit_carriers(ow)
                            continue
                        last_ldw = key
                    elif inst["engine"] == "PE" and op not in ("Matmult", "NoOp"):
                        last_ldw = None

                    if len(ow) > 1:
                        emit_carriers(ow[:-1])
                        si["on_wait"] = [ow[-1]]
                    out.append(inst)
                bb["instructions"] = out
        return json.dumps(d).encode()

    bass.Bass.to_json_bytes = patched


def build_kernel():
    nc = bass.Bass()
    xT = nc.declare_dram_parameter("xT", [DIM, SEQ], BF16, isOutput=False)
    wq = nc.declare_dram_parameter("wq", [DIM, FPC], BF16, isOutput=False)
    wk = nc.declare_dram_parameter("wk", [DIM, FPC], BF16, isOutput=False)
    wv = nc.declare_dram_parameter("wv", [DIM, FPC], BF16, isOutput=False)
    wo = nc.declare_dram_parameter("wo", [FPC, DIM], BF16, isOutput=False)
    # causal keep mask for the 128x128 diagonal block: msk[kl, ql] = kl <= ql
    msk = nc.declare_dram_parameter("msk", [128, 128], BF16, isOutput=False)
    outT = nc.declare_dram_parameter("outT", [DIM, SEQ], BF16, isOutput=True)

    with tile.TileContext(nc) as tc, ExitStack() as ctx:
        persist = ctx.enter_context(tc.tile_pool(name="persist", bufs=1))
        work = ctx.enter_context(tc.tile_pool(name="work", bufs=2))
        ps_mm = ctx.enter_context(tc.tile_pool(name="ps_mm", bufs=2, space="PSUM"))
        ps_s = ctx.enter_context(tc.tile_pool(name="ps_s", bufs=2, space="PSUM"))
        ps_o = ctx.enter_context(tc.tile_pool(name="ps_o", bufs=2, space="PSUM"))

        # ---- warm-up: keep PE busy (and HAM warming) during the DMA head --
        junk = persist.tile([128, 128], BF16, tag="junk", name="junk")
        nc.vector.memset(junk[:], 0.125)
        for i in range(30):
            pj = ps_mm.tile([128, 512], F32, tag="mm", name=f"warm{i}")
            nc.tensor.matmul(pj[:, 0:128], junk[:], junk[:],
                             start=True, stop=True)

        # ---- persistent tiles (create before the scoped weight pool) -----
        xT_sb = [persist.tile([128, SEQ], BF16, tag=f"xT{di}", name=f"xT{di}")
                 for di in range(8)]
        wo_sb = [persist.tile([128, DIM], BF16, tag=f"wo{fi}", name=f"wo{fi}")
                 for fi in range(4)]
        msk_sb = persist.tile([128, 128], BF16, tag="msk", name="msk")
        ones64 = persist.tile([1, DH], BF16, tag="ones64", name="ones64")
        qk_sb = {"q": [], "k": []}
        for qn in ("q", "k"):
            for fi in range(4):
                qk_sb[qn].append(
                    persist.tile([128, SEQ], BF16, tag=f"{qn}{fi}",
                                 name=f"{qn}{fi}"))
        v_sb = [persist.tile([128, HPC * VSTRIDE], BF16, tag=f"v{ti}",
                             name=f"v{ti}") for ti in range(NKT)]
        ot_sb = [persist.tile([128, SEQ], BF16, tag=f"ot{fi}", name=f"ot{fi}")
                 for fi in range(4)]

        # ---- input DMAs: xT on sync HWDGE, weights on act HWDGE ----------
        for di in range(8):
            nc.sync.dma_start(xT_sb[di][:], xT.ap()[di * 128:(di + 1) * 128, :])

        with tc.tile_pool(name="wpool", bufs=1) as wpool:
            w_sb = {"wq": [], "wk": [], "wv": []}
            for name, h in (("wq", wq), ("wk", wk), ("wv", wv)):
                for di in range(8):
                    t = wpool.tile([128, FPC], BF16, tag=f"{name}{di}",
                                   name=f"{name}{di}")
                    nc.scalar.dma_start(t[:], h.ap()[di * 128:(di + 1) * 128, :])
                    w_sb[name].append(t)
            for fi in range(4):
                nc.scalar.dma_start(wo_sb[fi][:],
                                    wo.ap()[fi * 128:(fi + 1) * 128, :])
            nc.scalar.dma_start(msk_sb[:], msk.ap()[:, :])
            nc.gpsimd.memset(ones64[:], 1.0)

            # ---- QKV projections (dual-pair chains keep PE dense while
            # the xT DMAs stream in) -------------------------------------
            def emit_qk_dual(qn, wn, fa, fb):
                sa = ps_s.tile([128, 1024], F32, tag="s", name="qk_sa")
                sb_ = ps_s.tile([128, 1024], F32, tag="s", name="qk_sb")
                ch = {
                    fa: [ps_mm.tile([128, 512], F32, tag="mm", name="cha0")[:],
                         ps_mm.tile([128, 512], F32, tag="mm", name="cha1")[:],
                         ps_o.tile([128, 512], F32, tag="o", name="cha2")[:],
                         ps_o.tile([128, 512], F32, tag="o", name="cha3")[:]],
                    fb: [sa[:, 0:512], sa[:, 512:1024],
                         sb_[:, 0:512], sb_[:, 512:1024]],
                }
                for di in range(8):
                    for fi in (fa, fb):
                        for tck in range(4):
                            nc.tensor.matmul(
                                ch[fi][tck],
                                w_sb[wn][di][:, fi * 128:(fi + 1) * 128],
                                xT_sb[di][:, tck * 512:(tck + 1) * 512],
                                start=(di == 0), stop=(di == 7),
                                skip_group_check=True)
                for fi in (fa, fb):
                    for tck in range(4):
                        nc.vector.tensor_copy(
                            qk_sb[qn][fi][:, tck * 512:(tck + 1) * 512],
                            ch[fi][tck])

            emit_qk_dual("q", "wq", 0, 1)
            emit_qk_dual("q", "wq", 2, 3)
            emit_qk_dual("k", "wk", 0, 1)
            emit_qk_dual("k", "wk", 2, 3)

            # ---- V for all 16 k-tiles (xT stationary, wv moving) ---------
            for ti in range(NKT):
                p = ps_mm.tile([128, 512], F32, tag="mm", name="p_v")
                for di in range(8):
                    nc.tensor.matmul(
                        p[:], xT_sb[di][:, ti * 128:(ti + 1) * 128],
                        w_sb["wv"][di][:],
                        start=(di == 0), stop=(di == 7))
                dst = v_sb[ti][:].rearrange("p (h c) -> p h c", h=HPC)[:, :, 0:DH]
                src = p[:].rearrange("p (h c) -> p h c", h=HPC)
                nc.scalar.copy(dst, src)
                nc.gpsimd.memset(
                    v_sb[ti][:].rearrange("p (h c) -> p h c", h=HPC)[:, :, DH:DH + 1],
                    1.0)
        # wpool closed: wq/wk/wv SBUF reclaimed for the pt pool below.

        pt_pool = ctx.enter_context(tc.tile_pool(name="pt", bufs=1))

        # ---- pipelined attention units ----------------------------------
        # unit u = (ci, pr). Per outer step: emit S matmuls + exps of u,
        # interleaving the PV matmuls of u-1 between them; normalization of
        # u-1 (rank-1 broadcast + ot write) is emitted early in step u+1.
        units = [(ci, pr) for ci in range(NQC) for pr in range(4)]
        state = {}  # per-unit tiles needed downstream

        def s_steps(u):
            """Yield per-j closures emitting S matmuls + exp (+ mask)."""
            ci, pr = u
            q0 = ci * 512
            pts = []
            state[u] = {"pts": pts}

            def make(j):
                def emit():
                    r = j - 4 * ci  # >=0 on the diagonal 4-block
                    rr = max(r, 0)
                    ps = ps_s.tile([128, 1024], F32, tag="s", name="ps_st")
                    for half in range(2):
                        nc.tensor.matmul(
                            ps[:, half * 512 + 128 * rr:(half + 1) * 512],
                            qk_sb["k"][pr][half * 64:(half + 1) * 64,
                                           j * 128:(j + 1) * 128],
                            qk_sb["q"][pr][half * 64:(half + 1) * 64,
                                           q0 + 128 * rr:q0 + 512],
                            start=True, stop=True, skip_group_check=True)
                    pt = pt_pool.tile([128, 1024], BF16, tag=f"pt{j}",
                                      name="pt", bufs=2)
                    pts.append(pt)
                    if r < 0:
                        nc.scalar.activation(pt[:], ps[:], Exp, scale=SCALE)
                    else:
                        pt3 = pt[:].rearrange(
                            "p (b w) -> p b w", b=2)[:, :, 128 * r:]
                        ps3 = ps[:].rearrange(
                            "p (b w) -> p b w", b=2)[:, :, 128 * r:]
                        nc.scalar.activation(pt3, ps3, Exp, scale=SCALE)
                        blk = pt[:].rearrange(
                            "p (b w) -> p b w", b=2)[:, :, 128 * r:128 * (r + 1)]
                        m3 = msk_sb[:][:, None, :].broadcast_to([128, 2, 128])
                        nc.vector.tensor_mul(blk, blk, m3)
                return emit
            return [make(j) for j in range(4 * ci + 4)]

        def pv_steps(u):
            """Yield closures emitting PV matmuls + po spill + recip of u."""
            ci, pr = u
            jn = 4 * ci + 4
            st = state[u]
            posb = work.tile([DH + 1, 1024], BF16, tag="posb", name="posb")
            st["posb"] = posb
            rrow_f = work.tile([1, 1024], F32, tag="rrowf", name="rrowf")
            rrow = work.tile([1, 1024], BF16, tag="rrow", name="rrow")
            st["rrow"] = rrow
            out = []
            for half in range(2):
                h = 2 * pr + half
                po = ps_o.tile([DH + 1, 512], F32, tag="o", name="po")

                def mk_mm(j, po=po, half=half, h=h):
                    def emit():
                        r = max(j - 4 * ci, 0)
                        nc.tensor.matmul(
                            po[:, 128 * r:512],
                            v_sb[j][:, h * VSTRIDE:h * VSTRIDE + DH + 1],
                            st["pts"][j][:, half * 512 + 128 * r:(half + 1) * 512],
                            start=(j == 0), stop=(j == jn - 1),
                            skip_group_check=True)
                    return emit
                out.extend(mk_mm(j) for j in range(jn))

                def mk_spill(po=po, half=half):
                    def emit():
                        nc.vector.tensor_copy(
                            posb[:, half * 512:(half + 1) * 512], po[:])
                    return emit
                out.append(mk_spill())

            def mk_recip():
                def emit():
                    nc.vector.reciprocal(rrow_f[:], posb[DH:DH + 1, :])
                    nc.vector.tensor_copy(rrow[:], rrow_f[:])
                return emit
            out.append(mk_recip())
            return out

        def emit_norm(u):
            """Rank-1 broadcast of the reciprocal row + normalized ot write."""
            ci, pr = u
            q0 = ci * 512
            st = state.pop(u)
            for half in range(2):
                row = half * 64
                rb_ps = ps_mm.tile([DH, 512], F32, tag="mm", name="rb_ps")
                nc.tensor.matmul(rb_ps[:], ones64[:],
                                 st["rrow"][:, half * 512:(half + 1) * 512],
                                 start=True, stop=True)
                rb = work.tile([DH, 512], BF16, tag="rb", name="rb")
                nc.vector.tensor_copy(rb[:], rb_ps[:])
                nc.vector.tensor_mul(
                    ot_sb[pr][row:row + 64, q0:q0 + 512],
                    st["posb"][0:DH, half * 512:(half + 1) * 512], rb[:])

        def emit_proj(ci):
            for ei in range(8):
                p = ps_mm.tile([128, 512], F32, tag="mm", name="p_proj")
                for fi in range(4):
                    nc.tensor.matmul(
                        p[:], wo_sb[fi][:, ei * 128:(ei + 1) * 128],
                        ot_sb[fi][:, ci * 512:(ci + 1) * 512],
                        start=(fi == 0), stop=(fi == 3))
                os_ = work.tile([128, 512], BF16, tag="os", name="os")
                nc.vector.tensor_copy(os_[:], p[:])
                nc.sync.dma_start(
                    outT.ap()[ei * 128:(ei + 1) * 128,
                              ci * 512:(ci + 1) * 512], os_[:])

        prev = None   # unit whose PV is being woven into the current step
        prev2 = None  # unit whose normalization is due
        for u in units:
            ci, pr = u
            ss = s_steps(u)
            pv = pv_steps(prev) if prev is not None else []
            # First two S j-steps, then the lagged normalization (its
            # reciprocal was queued near the end of the previous step).
            done_norm = False
            for idx, s in enumerate(ss):
                s()
                if idx == 1 and prev2 is not None:
                    emit_norm(prev2)
                    done_norm = True
                    if prev2[1] == 3:
                        emit_proj(prev2[0])
                # drain PV of prev proportionally across remaining S steps
                remaining_s = len(ss) - idx - 1
                if pv:
                    take = -(-len(pv) // (remaining_s + 1))
                    for _ in range(take):
                        pv.pop(0)()
            for w_ in pv:
                w_()
            if prev2 is not None and not done_norm:
                emit_norm(prev2)
                if prev2[1] == 3:
                    emit_proj(prev2[0])
            prev2 = prev
            prev = u
        # drain tail: PV of the last unit, then its normalization + proj
        for w_ in pv_steps(prev):
            w_()
        emit_norm(prev2)
        emit_norm(prev)
        emit_proj(NQC - 1)
    return nc


_NC = None


def _get_nc():
    global _NC
    if _NC is None:
        _patch_walrus_wait_limit()
        _NC = build_kernel()
    return _NC


def _host_mask():
    kl = np.arange(128)[:, None]
    ql = np.arange(128)[None, :]
    return (kl <= ql).astype(ml_dtypes.bfloat16)


def kernel(x, w_qkv, w_out, _trace=False, _trace_kwargs=None):
    x = np.asarray(x, dtype=np.float32)
    w_qkv = np.asarray(w_qkv, dtype=np.float32)
    w_out = np.asarray(w_out, dtype=np.float32)
    nc = _get_nc()

    msk = _host_mask()
    xTb = [np.ascontiguousarray(x[b].T).astype(ml_dtypes.bfloat16)
           for b in range(4)]
    in_maps = []
    for c in range(NCORES):
        b, g = c // 2, c % 2
        cols = slice(g * FPC, (g + 1) * FPC)
        in_maps.append({
            "xT": xTb[b],
            "wq": w_qkv[:, 0 * DIM:1 * DIM][:, cols].astype(ml_dtypes.bfloat16),
            "wk": w_qkv[:, 1 * DIM:2 * DIM][:, cols].astype(ml_dtypes.bfloat16),
            "wv": w_qkv[:, 2 * DIM:3 * DIM][:, cols].astype(ml_dtypes.bfloat16),
            "wo": w_out[g * FPC:(g + 1) * FPC, :].astype(ml_dtypes.bfloat16),
            "msk": msk,
        })

    res = run_bass_kernel_spmd(
        nc, in_maps, core_ids=list(range(NCORES)),
        trace=_trace, **(_trace_kwargs or {}))
    out = np.empty((4, SEQ, DIM), dtype=np.float32)
    for b in range(4):
        out[b] = (res.results[2 * b]["outT"].astype(np.float32)
                  + res.results[2 * b + 1]["outT"].astype(np.float32)).T
    if _trace:
        kernel.last_results = res
    return out
